# revision 11
# baseline (speedup 1.0000x reference)
"""Trainium2 Bass kernel for nn_AttentionSpatial (spatial cosine attention).

Math (per head h of 8):
  q = w_q @ X, k/v = w_kv @ Y          (1x1 convs == channel matmuls)
  qn = l2norm(q) * temp, kn = l2norm(k)  (norm over the 8 head channels)
  S^T[m, n] = kn_m . qn_n               (keys on partitions, queries on free)
  P = exp(S^T)                          (logits bounded by |temp| => no max pass)
  [O; den] = [V | 1]^T @ P              (attn @ v with fused softmax denominator)
  partial = (w_out[:, 8h:8h+8] @ O) / den
Full output = sum over heads of partials (host-side reduce over the 8 cores).

Sharding: head h -> core h. Each core gets full X, Y and its head's weight
slices; returns a [64, 4096] partial of the final projection.
"""

import numpy as np

import concourse.bass as bass
import concourse.tile as tile
from concourse import mybir
from concourse.masks import make_identity
from concourse.vector_clock import ScopedClock

NUM_HEADS = 8
DIM = 64          # channels
HD = 8            # head dim
N = 4096          # tokens (h*w)
NB = 32           # 128-token blocks
QC = 512          # query chunk
NQC = N // QC
F32 = mybir.dt.float32
F32R = mybir.dt.float32r

# kb waves for the exp stage: 3 PSUM banks per wave (double buffered) so the
# activation engine runs one large-FD exp per wave.
WAVES = [list(range(s, min(s + 3, NB))) for s in range(0, NB, 3)]

_patched = False


def _apply_walrus_compat():
    """This container's walrus build rejects Drain instructions that carry
    sync waits ("Too many sync wait commands").  Replace multi-engine
    barriers with the sem-only variant and re-emit the TileContext tail
    drain's waits as standalone EventSemaphore instructions."""
    global _patched
    if _patched:
        return
    _patched = True

    def meb(self, engines):
        for e in engines:
            self.engines[e].drain()  # bare drain: flush pipelines, no waits
        for inst in self._sem_only_all_engine_barrier_insts("meb"):
            self.engines[inst.engine].add_instruction(inst)

    bass.Bass.multi_engine_barrier = meb

    def _drain_and_barrier(self, tick_clock, wait_clock):
        nc = self.nc
        carrier = nc.sync.nop()
        wait_clock.add_sem_waits(
            carrier.ins, ScopedClock({None: tick_clock.global_clock})
        )
        si = carrier.ins.sync_info
        waits = list(si.on_wait) if si and si.on_wait else []
        if si is not None:
            si.on_wait = []
        sems = list(self.sems.allocated().values())
        placeholder = sems[0] if sems else nc.alloc_semaphore("tailw")
        for w in waits:
            assert w.wait_mode in ("sem-ge-imm", "sem-ge"), w.wait_mode
            ev = nc.sync.wait_ge(placeholder, 0)
            ev.ins.sync_info.on_wait = [w]
        nc.sync.drain()
        nc.all_engine_barrier()
        popped = nc._tile_sem_poison_stack.pop()
        assert popped is self._sem_poison
        nc.clear_and_free_semaphores(list(self.sems.allocated().values()))
        nc.all_engine_barrier()

    tile.TileContext._drain_and_barrier = _drain_and_barrier

    # This walrus build allows at most ONE sync-wait command per instruction
    # (and none on Drain).  Split extra waits into standalone single-wait
    # EventSemaphore instructions emitted just before, on the same engine.
    orig_commit = tile.TileContext._commit_instruction

    def _commit_instruction(self, inst, lazy_reg_writes=True):
        si = inst.sync_info
        if si is not None and si.on_wait:
            is_drain = type(inst).__name__ == "InstDrain"
            waits = list(si.on_wait)
            n_ge = sum(
                1 for w in waits if w.wait_mode in ("sem-ge-imm", "sem-ge")
            )
            assert n_ge == len(waits) or not is_drain, f"eq-wait on drain {inst}"
            keep = 0 if is_drain else 1
            if len(waits) > keep and inst.engine != mybir.EngineType.Unassigned:
                kept, split = waits[:keep], waits[keep:]
                si.on_wait = kept
                sems = list(self.sems.allocated().values())
                placeholder = sems[0] if sems else self.nc.alloc_semaphore("splitw")
                eng = self.nc.engines[inst.engine]
                for w in split:
                    assert w.wait_mode in ("sem-ge-imm", "sem-ge"), w.wait_mode
                    ev = eng.wait_ge(placeholder, 0)
                    ev.ins.sync_info.on_wait = [w]
        return orig_commit(self, inst, lazy_reg_writes)

    tile.TileContext._commit_instruction = _commit_instruction


def _emit_head_attention(tc, rep, x_d, y_d, wqkvt_d, wot_d, temp_d, out_d):
    """Emit one head's full attention for one repetition."""
    import contextlib

    nc = tc.nc
    Exp = mybir.ActivationFunctionType.Exp
    Sqrt = mybir.ActivationFunctionType.Sqrt

    ctx = contextlib.ExitStack()
    with ctx:
        const = ctx.enter_context(tc.tile_pool(name=f"const{rep}", bufs=1))
        sb = ctx.enter_context(tc.tile_pool(name=f"sb{rep}", bufs=1))

        # ---- load inputs ----
        X = const.tile([DIM, N], F32)
        Y = const.tile([DIM, N], F32)
        W = const.tile([DIM, 3 * HD], F32)
        WOT = const.tile([HD, DIM], F32)
        WOTR = const.tile([HD, DIM], F32R)
        nc.sync.dma_start(X[:], x_d[:])
        nc.sync.dma_start(Y[:], y_d[:])
        nc.sync.dma_start(W[:], wqkvt_d[:])
        nc.sync.dma_start(WOT[:], wot_d[:])
        nc.vector.tensor_copy(WOTR[:], WOT[:])
        tmp_bc = const.tile([128, 1], F32)
        nc.gpsimd.dma_start(
            out=tmp_bc[:],
            in_=bass.AP(
                tensor=temp_d.tensor, offset=temp_d.offset, ap=[[0, 128], [1, 1]]
            ),
        )
        ident = const.tile([128, 128], F32)
        make_identity(nc, ident[:])

        # persistent SBUF state for this head
        QK = sb.tile([128, NB, 2 * HD], F32)   # token-major q|k
        Vaug = sb.tile([128, NB, HD + 1], F32R)  # token-major v | ones
        Qcm = sb.tile([HD, N], F32R)            # channel-major normalized q
        Kcm = sb.tile([HD, N], F32R)            # channel-major normalized k
        rqk = sb.tile([128, 2 * NB], F32)      # per-token 1/norm (q | k)

        ones32 = const.tile([128, NB], F32)
        nc.vector.memset(ones32[:], 1.0)
        nc.vector.tensor_copy(
            Vaug[:, :, HD : HD + 1], ones32[:].rearrange("p (a b) -> p a b", b=1)
        )

        # ---- step 1: token-major projections  psum[128t, 24] per block ----
        with tc.tile_pool(name=f"pproj{rep}", bufs=4, space="PSUM") as pproj:
            for i in range(NB):
                ps = pproj.tile([128, 3 * HD], F32)
                nc.tensor.matmul(
                    ps[:, 0:HD],
                    lhsT=X[:, i * 128 : (i + 1) * 128],
                    rhs=W[:, 0:HD],
                    start=True,
                    stop=True,
                )
                nc.tensor.matmul(
                    ps[:, HD : 3 * HD],
                    lhsT=Y[:, i * 128 : (i + 1) * 128],
                    rhs=W[:, HD : 3 * HD],
                    start=True,
                    stop=True,
                )
                nc.vector.tensor_copy(QK[:, i, :], ps[:, 0 : 2 * HD])
                nc.scalar.copy(Vaug[:, i, 0:HD], ps[:, 2 * HD : 3 * HD])

        # ---- step 2: per-token L2 norms (token-major) ----
        sq = sb.tile([128, NB, HD], F32)
        nrm = sb.tile([128, 2 * NB], F32)
        nc.vector.tensor_mul(sq[:], QK[:, :, 0:HD], QK[:, :, 0:HD])
        nc.vector.tensor_reduce(
            nrm[:, 0:NB], sq[:], axis=mybir.AxisListType.X, op=mybir.AluOpType.add
        )
        nc.vector.tensor_mul(sq[:], QK[:, :, HD : 2 * HD], QK[:, :, HD : 2 * HD])
        nc.vector.tensor_reduce(
            nrm[:, NB : 2 * NB],
            sq[:],
            axis=mybir.AxisListType.X,
            op=mybir.AluOpType.add,
        )
        # norm = sqrt(ssq); rqk = 1/norm; fold temperature into q's.
        # (reference clamps the norm at 1e-12 — unreachable for randn data)
        nc.scalar.activation(nrm[:], nrm[:], Sqrt, bias=0.0)
        nc.vector.reciprocal(rqk[:], nrm[:])
        nc.vector.tensor_scalar_mul(rqk[:, 0:NB], in0=rqk[:, 0:NB], scalar1=tmp_bc[:])

        # ---- step 3: normalize q, k in token-major layout ----
        for i in range(NB):
            nc.vector.tensor_scalar_mul(
                QK[:, i, 0:HD], in0=QK[:, i, 0:HD], scalar1=rqk[:, i : i + 1]
            )
            nc.vector.tensor_scalar_mul(
                QK[:, i, HD : 2 * HD],
                in0=QK[:, i, HD : 2 * HD],
                scalar1=rqk[:, NB + i : NB + i + 1],
            )

        # ---- step 4: transpose to channel-major via the tensor engine ----
        with tc.tile_pool(name=f"ptr{rep}", bufs=4, space="PSUM") as ptr:
            for half, dst in ((0, Qcm), (1, Kcm)):
                for g in range(NB // 4):  # 4 blocks -> one [8, 512] psum bank
                    pt = ptr.tile([HD, 512], F32)
                    for j in range(4):
                        i = g * 4 + j
                        nc.tensor.transpose(
                            pt[:, j * 128 : (j + 1) * 128],
                            QK[:, i, half * HD : (half + 1) * HD],
                            ident[:],
                        )
                    eng = nc.vector if (g % 2 == 0) else nc.scalar
                    if eng is nc.vector:
                        nc.vector.tensor_copy(
                            dst[:, g * 512 : (g + 1) * 512], pt[:]
                        )
                    else:
                        nc.scalar.copy(dst[:, g * 512 : (g + 1) * 512], pt[:])

        # ---- main loop ----
        pS = ctx.enter_context(tc.tile_pool(name=f"pS{rep}", bufs=2, space="PSUM"))
        pO = ctx.enter_context(tc.tile_pool(name=f"pO{rep}", bufs=1, space="PSUM"))
        pF = ctx.enter_context(tc.tile_pool(name=f"pF{rep}", bufs=1, space="PSUM"))
        Ppool = ctx.enter_context(tc.tile_pool(name=f"P{rep}", bufs=2))
        epi = ctx.enter_context(tc.tile_pool(name=f"epi{rep}", bufs=2))

        for qc in range(NQC):
            q_sl = Qcm[:, qc * QC : (qc + 1) * QC]
            O = pO.tile([HD + 1, QC], F32)
            for wave in WAVES:
                nw = len(wave)
                S = pS.tile([128, 3 * QC], F32, tag="S")
                P = Ppool.tile([128, 3 * QC], F32R, tag="P")
                for j, kb in enumerate(wave):
                    nc.tensor.matmul(
                        S[:, j * QC : (j + 1) * QC],
                        lhsT=Kcm[:, kb * 128 : (kb + 1) * 128],
                        rhs=q_sl,
                        start=True,
                        stop=True,
                    )
                nc.scalar.activation(
                    P[:, 0 : nw * QC], S[:, 0 : nw * QC], Exp
                )
                for j, kb in enumerate(wave):
                    nc.tensor.matmul(
                        O[:],
                        lhsT=Vaug[:, kb, :],
                        rhs=P[:, j * QC : (j + 1) * QC],
                        start=(kb == 0),
                        stop=(kb == NB - 1),
                    )

            # epilogue: projection + softmax division
            O_sb = epi.tile([HD + 1, QC], F32R, tag="O_sb")
            nc.vector.tensor_copy(O_sb[:], O[:])
            den0 = epi.tile([1, QC], F32, tag="den0")
            nc.sync.dma_start(den0[:], O_sb[HD : HD + 1, :].bitcast(F32))
            rden0 = epi.tile([1, QC], F32, tag="rden0")
            nc.vector.reciprocal(rden0[:], den0[:])
            rden = epi.tile([DIM, QC], F32, tag="rden")
            r0 = rden0[:]
            nc.sync.dma_start(
                rden[:],
                bass.AP(
                    tensor=r0.tensor,
                    offset=r0.offset,
                    ap=[[1, 1], [0, DIM], [1, QC]],
                ),
            )
            proj = pF.tile([DIM, QC], F32)
            nc.tensor.matmul(
                proj[:],
                lhsT=WOTR[:],
                rhs=O_sb[0:HD, :],
                start=True,
                stop=True,
            )
            res = epi.tile([DIM, QC], F32, tag="res")
            nc.vector.tensor_mul(res[:], proj[:], rden[:])
            nc.sync.dma_start(out_d[:, qc * QC : (qc + 1) * QC], res[:])


def build_program(reps: int = 1):
    """Build the SPMD bass program (identical on all cores)."""
    _apply_walrus_compat()
    nc = bass.Bass("TRN2", target_bir_lowering=False, debug=False)
    x_d = nc.dram_tensor("x", [DIM, N], F32, kind="ExternalInput").ap()
    y_d = nc.dram_tensor("y", [DIM, N], F32, kind="ExternalInput").ap()
    wqkvt_d = nc.dram_tensor("wqkvt", [DIM, 3 * HD], F32, kind="ExternalInput").ap()
    wot_d = nc.dram_tensor("wot", [HD, DIM], F32, kind="ExternalInput").ap()
    temp_d = nc.dram_tensor("temp", [1, 1], F32, kind="ExternalInput").ap()
    outs = []
    with tile.TileContext(nc) as tc:
        for rep in range(reps):
            out_d = nc.dram_tensor(
                f"out{rep}", [DIM, N], F32, kind="ExternalOutput"
            ).ap()
            outs.append(f"out{rep}")
            _emit_head_attention(tc, rep, x_d, y_d, wqkvt_d, wot_d, temp_d, out_d)
    return nc, outs


def make_in_maps(x, y, w_q, w_kv, w_out, temperature):
    x = np.ascontiguousarray(np.asarray(x, dtype=np.float32))
    y = np.ascontiguousarray(np.asarray(y, dtype=np.float32))
    w_q = np.asarray(w_q, dtype=np.float32)
    w_kv = np.asarray(w_kv, dtype=np.float32)
    w_out = np.asarray(w_out, dtype=np.float32)
    temperature = np.asarray(temperature, dtype=np.float32)
    assert x.shape == (1, DIM, 64, 64) and y.shape == (1, DIM, 64, 64)
    X = x.reshape(DIM, N)
    Y = y.reshape(DIM, N)
    in_maps = []
    for h in range(NUM_HEADS):
        sl = slice(h * HD, (h + 1) * HD)
        wqkvt = np.concatenate(
            [w_q[sl].T, w_kv[sl].T, w_kv[DIM + h * HD : DIM + (h + 1) * HD].T],
            axis=1,
        )
        in_maps.append(
            {
                "x": X,
                "y": Y,
                "wqkvt": np.ascontiguousarray(wqkvt),
                "wot": np.ascontiguousarray(w_out[:, sl].T),
                "temp": temperature.reshape(NUM_HEADS)[h].reshape(1, 1),
            }
        )
    return in_maps


def kernel(x, y, w_q, w_kv, w_out, temperature):
    from concourse.bass_utils import run_bass_kernel_spmd

    nc, out_names = build_program(reps=1)
    in_maps = make_in_maps(x, y, w_q, w_kv, w_out, temperature)
    res = run_bass_kernel_spmd(nc, in_maps, list(range(NUM_HEADS)))
    total = np.zeros((DIM, N), dtype=np.float32)
    for h in range(NUM_HEADS):
        total += res.results[h][out_names[0]]
    return total.reshape(1, DIM, 64, 64)


# revision 18
# speedup vs baseline: 1.2903x; 1.2903x over previous
"""Trainium2 Bass kernel for nn_AttentionSpatial (spatial cosine attention).

Math (per head h of 8):
  q = w_q @ X, k/v = w_kv @ Y          (1x1 convs == channel matmuls)
  qn = l2norm(q) * temp, kn = l2norm(k)  (norm over the 8 head channels)
  S^T[m, n] = kn_m . qn_n               (keys on partitions, queries on free)
  P = exp(S^T)                          (logits bounded by |temp| => no max pass)
  [O; den] = [V | 1]^T @ P              (attn @ v with fused softmax denominator)
  partial = (w_out[:, 8h:8h+8] @ O) / den
Full output = sum over heads of partials (host-side reduce over the 8 cores).

Sharding: head h -> core h. Each core gets full X, Y and its head's weight
slices; returns a [64, 4096] partial of the final projection.
"""

import numpy as np

import concourse.bass as bass
import concourse.tile as tile
from concourse import mybir
from concourse.masks import make_identity
from concourse.vector_clock import ScopedClock

NUM_HEADS = 8
DIM = 64          # channels
HD = 8            # head dim
N = 4096          # tokens (h*w)
NB = 32           # 128-token blocks
QC = 512          # query chunk
NQC = N // QC
F32 = mybir.dt.float32
F32R = mybir.dt.float32r

# kb waves for the exp stage: 3 PSUM banks per wave (double buffered) so the
# activation engine runs one large-FD exp per wave.
WAVES = [list(range(s, min(s + 3, NB))) for s in range(0, NB, 3)]

import os
ROW_PACK_S = os.environ.get("KERN_ROW_PACK", "1") == "1"
COL_PACK_O = os.environ.get("KERN_COL_PACK", "0") == "1" 

_patched = False


def _apply_walrus_compat():
    """This container's walrus build rejects Drain instructions that carry
    sync waits ("Too many sync wait commands").  Replace multi-engine
    barriers with the sem-only variant and re-emit the TileContext tail
    drain's waits as standalone EventSemaphore instructions."""
    global _patched
    if _patched:
        return
    _patched = True

    def meb(self, engines):
        for e in engines:
            self.engines[e].drain()  # bare drain: flush pipelines, no waits
        for inst in self._sem_only_all_engine_barrier_insts("meb"):
            self.engines[inst.engine].add_instruction(inst)

    bass.Bass.multi_engine_barrier = meb

    def _drain_and_barrier(self, tick_clock, wait_clock):
        nc = self.nc
        carrier = nc.sync.nop()
        wait_clock.add_sem_waits(
            carrier.ins, ScopedClock({None: tick_clock.global_clock})
        )
        si = carrier.ins.sync_info
        waits = list(si.on_wait) if si and si.on_wait else []
        if si is not None:
            si.on_wait = []
        sems = list(self.sems.allocated().values())
        placeholder = sems[0] if sems else nc.alloc_semaphore("tailw")
        for w in waits:
            assert w.wait_mode in ("sem-ge-imm", "sem-ge"), w.wait_mode
            ev = nc.sync.wait_ge(placeholder, 0)
            ev.ins.sync_info.on_wait = [w]
        nc.sync.drain()
        nc.all_engine_barrier()
        popped = nc._tile_sem_poison_stack.pop()
        assert popped is self._sem_poison
        nc.clear_and_free_semaphores(list(self.sems.allocated().values()))
        nc.all_engine_barrier()

    tile.TileContext._drain_and_barrier = _drain_and_barrier

    # This walrus build allows at most ONE sync-wait command per instruction
    # (and none on Drain).  Split extra waits into standalone single-wait
    # EventSemaphore instructions emitted just before, on the same engine.
    orig_commit = tile.TileContext._commit_instruction

    def _commit_instruction(self, inst, lazy_reg_writes=True):
        si = inst.sync_info
        if si is not None and si.on_wait:
            is_drain = type(inst).__name__ == "InstDrain"
            waits = list(si.on_wait)
            n_ge = sum(
                1 for w in waits if w.wait_mode in ("sem-ge-imm", "sem-ge")
            )
            assert n_ge == len(waits) or not is_drain, f"eq-wait on drain {inst}"
            keep = 0 if is_drain else 1
            if len(waits) > keep and inst.engine != mybir.EngineType.Unassigned:
                kept, split = waits[:keep], waits[keep:]
                si.on_wait = kept
                sems = list(self.sems.allocated().values())
                placeholder = sems[0] if sems else self.nc.alloc_semaphore("splitw")
                eng = self.nc.engines[inst.engine]
                for w in split:
                    assert w.wait_mode in ("sem-ge-imm", "sem-ge"), w.wait_mode
                    ev = eng.wait_ge(placeholder, 0)
                    ev.ins.sync_info.on_wait = [w]
        return orig_commit(self, inst, lazy_reg_writes)

    tile.TileContext._commit_instruction = _commit_instruction


def _emit_head_attention(tc, rep, x_d, y_d, wqkvt_d, wot_d, temp_d, out_d):
    """Emit one head's full attention for one repetition."""
    import contextlib

    nc = tc.nc
    Exp = mybir.ActivationFunctionType.Exp
    Sqrt = mybir.ActivationFunctionType.Sqrt

    ctx = contextlib.ExitStack()
    with ctx:
        const = ctx.enter_context(tc.tile_pool(name=f"const{rep}", bufs=1))
        sb = ctx.enter_context(tc.tile_pool(name=f"sb{rep}", bufs=1))

        # ---- load inputs ----
        X = const.tile([DIM, N], F32)
        Y = const.tile([DIM, N], F32)
        W = const.tile([DIM, 3 * HD], F32)
        WOT = const.tile([128, DIM + 1], F32)
        WOTR = const.tile([128, DIM + 1], F32R)
        nc.sync.dma_start(X[:], x_d[:])
        nc.sync.dma_start(Y[:], y_d[:])
        nc.sync.dma_start(W[:], wqkvt_d[:])
        nc.sync.dma_start(WOT[:], wot_d[:])
        nc.vector.tensor_copy(WOTR[:], WOT[:])
        tmp_bc = const.tile([128, 1], F32)
        nc.gpsimd.dma_start(
            out=tmp_bc[:],
            in_=bass.AP(
                tensor=temp_d.tensor, offset=temp_d.offset, ap=[[0, 128], [1, 1]]
            ),
        )
        ident = const.tile([128, 128], F32)
        make_identity(nc, ident[:])

        # persistent SBUF state for this head
        QK = sb.tile([128, NB, 2 * HD], F32)   # token-major q|k
        Vaug = sb.tile([128, NB, HD + 1], F32R)  # token-major v | ones
        # channel-major normalized q/k, replicated at partitions 0/32/64 for
        # row-group-packed S matmuls
        Qcm = sb.tile([72, N], F32R)
        Kcm = sb.tile([72, N], F32R)
        rqk = sb.tile([128, 2 * NB], F32)      # per-token 1/norm (q | k)

        ones32 = const.tile([128, NB], F32)
        nc.vector.memset(ones32[:], 1.0)
        nc.vector.tensor_copy(
            Vaug[:, :, HD : HD + 1], ones32[:].rearrange("p (a b) -> p a b", b=1)
        )

        # ---- step 1: token-major projections  psum[128t, 24] per block ----
        with tc.tile_pool(name=f"pproj{rep}", bufs=4, space="PSUM") as pproj:
            for i in range(NB):
                ps = pproj.tile([128, 3 * HD], F32)
                nc.tensor.matmul(
                    ps[:, 0:HD],
                    lhsT=X[:, i * 128 : (i + 1) * 128],
                    rhs=W[:, 0:HD],
                    start=True,
                    stop=True,
                )
                nc.tensor.matmul(
                    ps[:, HD : 3 * HD],
                    lhsT=Y[:, i * 128 : (i + 1) * 128],
                    rhs=W[:, HD : 3 * HD],
                    start=True,
                    stop=True,
                )
                nc.vector.tensor_copy(QK[:, i, :], ps[:, 0 : 2 * HD])
                nc.vector.tensor_copy(Vaug[:, i, 0:HD], ps[:, 2 * HD : 3 * HD])

        # ---- step 2: per-token L2 norms (token-major) ----
        sq = sb.tile([128, NB, HD], F32)
        nrm = sb.tile([128, 2 * NB], F32)
        nc.vector.tensor_mul(sq[:], QK[:, :, 0:HD], QK[:, :, 0:HD])
        nc.vector.tensor_reduce(
            nrm[:, 0:NB], sq[:], axis=mybir.AxisListType.X, op=mybir.AluOpType.add
        )
        nc.vector.tensor_mul(sq[:], QK[:, :, HD : 2 * HD], QK[:, :, HD : 2 * HD])
        nc.vector.tensor_reduce(
            nrm[:, NB : 2 * NB],
            sq[:],
            axis=mybir.AxisListType.X,
            op=mybir.AluOpType.add,
        )
        # norm = sqrt(ssq); rqk = 1/norm; fold temperature into q's.
        # (reference clamps the norm at 1e-12 — unreachable for randn data)
        nc.scalar.activation(nrm[:], nrm[:], Sqrt, bias=0.0)
        nc.vector.reciprocal(rqk[:], nrm[:])
        nc.vector.tensor_scalar_mul(rqk[:, 0:NB], in0=rqk[:, 0:NB], scalar1=tmp_bc[:])

        # ---- step 3: normalize q, k in token-major layout ----
        for i in range(NB):
            nc.vector.tensor_scalar_mul(
                QK[:, i, 0:HD], in0=QK[:, i, 0:HD], scalar1=rqk[:, i : i + 1]
            )
            nc.vector.tensor_scalar_mul(
                QK[:, i, HD : 2 * HD],
                in0=QK[:, i, HD : 2 * HD],
                scalar1=rqk[:, NB + i : NB + i + 1],
            )

        # ---- step 4: transpose to channel-major via the tensor engine ----
        with tc.tile_pool(name=f"ptr{rep}", bufs=4, space="PSUM") as ptr:
            for half, dst in ((0, Qcm), (1, Kcm)):
                for g in range(NB // 4):  # 4 blocks -> one [8, 512] psum bank
                    pt = ptr.tile([HD, 512], F32)
                    for j in range(4):
                        i = g * 4 + j
                        nc.tensor.transpose(
                            pt[:, j * 128 : (j + 1) * 128],
                            QK[:, i, half * HD : (half + 1) * HD],
                            ident[:],
                        )
                    nc.vector.tensor_copy(
                        dst[0:HD, g * 512 : (g + 1) * 512], pt[:]
                    )

        # replicate q/k to partitions 32-39 and 64-71 (row groups 1, 2)
        for t in (Qcm, Kcm):
            for base in (32, 64):
                nc.sync.dma_start(
                    t[base : base + HD, :].bitcast(F32),
                    t[0:HD, :].bitcast(F32),
                )

        # ---- main loop ----
        pS = ctx.enter_context(tc.tile_pool(name=f"pS{rep}", bufs=2, space="PSUM"))
        pO = ctx.enter_context(tc.tile_pool(name=f"pO{rep}", bufs=1, space="PSUM"))
        pF = ctx.enter_context(tc.tile_pool(name=f"pF{rep}", bufs=1, space="PSUM"))
        Ppool = ctx.enter_context(tc.tile_pool(name=f"P{rep}", bufs=2))
        epi = ctx.enter_context(tc.tile_pool(name=f"epi{rep}", bufs=2))
        dram = ctx.enter_context(
            tc.tile_pool(name=f"dram{rep}", bufs=2, space="DRAM")
        )

        for qc in range(NQC):
            O = pO.tile([128, QC], F32)
            # zero the rows between the 4 accumulator groups so the epilogue
            # copy reads fully initialized memory
            nc.vector.memset(O[:], 0.0)
            for wave in WAVES:
                nw = len(wave)
                S = pS.tile([128, 3 * QC], F32, tag="S")
                P = Ppool.tile([128, 3 * QC], F32R, tag="P")
                for j, kb in enumerate(wave):
                    b = 32 * j if ROW_PACK_S else 0  # row group j
                    nc.tensor.matmul(
                        S[:, j * QC : (j + 1) * QC],
                        lhsT=Kcm[b : b + HD, kb * 128 : (kb + 1) * 128],
                        rhs=Qcm[b : b + HD, qc * QC : (qc + 1) * QC],
                        start=True,
                        stop=True,
                    )
                nc.scalar.activation(
                    P[:, 0 : nw * QC], S[:, 0 : nw * QC], Exp
                )
                for j, kb in enumerate(wave):
                    g = (kb % 4) if COL_PACK_O else 0
                    nc.tensor.matmul(
                        O[32 * g : 32 * g + HD + 1, :],
                        lhsT=Vaug[:, kb, :],
                        rhs=P[:, j * QC : (j + 1) * QC],
                        start=(kb < 4) if COL_PACK_O else (kb == 0),
                        stop=(kb >= NB - 4) if COL_PACK_O else (kb == NB - 1),
                        tile_position=(0, 32 * g) if COL_PACK_O else None,
                        skip_group_check=True,
                    )

            # epilogue: one matmul projects + merges the 4 accumulators and
            # sums the denominator rows (WOT4 col 64 selects them)
            O_sb = epi.tile([128, QC], F32R, tag="O_sb")
            nc.vector.tensor_copy(O_sb[:], O[:])
            proj = pF.tile([DIM + 1, QC], F32)
            nc.tensor.matmul(
                proj[:],
                lhsT=WOTR[:],
                rhs=O_sb[:],
                start=True,
                stop=True,
            )
            rden0 = epi.tile([DIM + 1, QC], F32, tag="rden0")
            nc.vector.reciprocal(rden0[DIM : DIM + 1, :], proj[DIM : DIM + 1, :])
            dscr = dram.tile([1, QC], F32, tag="dscr")
            nc.sync.dma_start(dscr[:], rden0[DIM : DIM + 1, :])
            rden = epi.tile([DIM, QC], F32, tag="rden")
            d0 = dscr[:]
            nc.sync.dma_start(
                rden[:],
                bass.AP(tensor=d0.tensor, offset=d0.offset, ap=[[0, DIM], [1, QC]]),
            )
            res = epi.tile([DIM, QC], F32, tag="res")
            nc.vector.tensor_mul(res[:], proj[0:DIM, :], rden[:])
            nc.sync.dma_start(out_d[:, qc * QC : (qc + 1) * QC], res[:])


def build_program(reps: int = 1):
    """Build the SPMD bass program (identical on all cores)."""
    _apply_walrus_compat()
    nc = bass.Bass("TRN2", target_bir_lowering=False, debug=False)
    x_d = nc.dram_tensor("x", [DIM, N], F32, kind="ExternalInput").ap()
    y_d = nc.dram_tensor("y", [DIM, N], F32, kind="ExternalInput").ap()
    wqkvt_d = nc.dram_tensor("wqkvt", [DIM, 3 * HD], F32, kind="ExternalInput").ap()
    wot_d = nc.dram_tensor("wot", [128, DIM + 1], F32, kind="ExternalInput").ap()
    temp_d = nc.dram_tensor("temp", [1, 1], F32, kind="ExternalInput").ap()
    outs = []
    with tile.TileContext(nc) as tc:
        for rep in range(reps):
            out_d = nc.dram_tensor(
                f"out{rep}", [DIM, N], F32, kind="ExternalOutput"
            ).ap()
            outs.append(f"out{rep}")
            _emit_head_attention(tc, rep, x_d, y_d, wqkvt_d, wot_d, temp_d, out_d)
    return nc, outs


def make_in_maps(x, y, w_q, w_kv, w_out, temperature):
    x = np.ascontiguousarray(np.asarray(x, dtype=np.float32))
    y = np.ascontiguousarray(np.asarray(y, dtype=np.float32))
    w_q = np.asarray(w_q, dtype=np.float32)
    w_kv = np.asarray(w_kv, dtype=np.float32)
    w_out = np.asarray(w_out, dtype=np.float32)
    temperature = np.asarray(temperature, dtype=np.float32)
    assert x.shape == (1, DIM, 64, 64) and y.shape == (1, DIM, 64, 64)
    X = x.reshape(DIM, N)
    Y = y.reshape(DIM, N)
    in_maps = []
    for h in range(NUM_HEADS):
        sl = slice(h * HD, (h + 1) * HD)
        wqkvt = np.concatenate(
            [w_q[sl].T, w_kv[sl].T, w_kv[DIM + h * HD : DIM + (h + 1) * HD].T],
            axis=1,
        )
        # cols 0..64 project the 4 accumulator groups; col 64 sums their
        # denominator rows
        wot4 = np.zeros((128, DIM + 1), dtype=np.float32)
        for g in range(4):
            wot4[32 * g : 32 * g + HD, 0:DIM] = w_out[:, sl].T
            wot4[32 * g + HD, DIM] = 1.0
        in_maps.append(
            {
                "x": X,
                "y": Y,
                "wqkvt": np.ascontiguousarray(wqkvt),
                "wot": wot4,
                "temp": temperature.reshape(NUM_HEADS)[h].reshape(1, 1),
            }
        )
    return in_maps


def kernel(x, y, w_q, w_kv, w_out, temperature):
    from concourse.bass_utils import run_bass_kernel_spmd

    nc, out_names = build_program(reps=1)
    in_maps = make_in_maps(x, y, w_q, w_kv, w_out, temperature)
    res = run_bass_kernel_spmd(nc, in_maps, list(range(NUM_HEADS)))
    total = np.zeros((DIM, N), dtype=np.float32)
    for h in range(NUM_HEADS):
        total += res.results[h][out_names[0]]
    return total.reshape(1, DIM, 64, 64)


# revision 22
# speedup vs baseline: 703.1198x; 544.9162x over previous
"""Trainium2 Bass kernel for nn_AttentionSpatial (spatial cosine attention).

Math (per head h of 8):
  q = w_q @ X, k/v = w_kv @ Y          (1x1 convs == channel matmuls)
  qn = l2norm(q) * temp, kn = l2norm(k)  (norm over the 8 head channels)
  S^T[m, n] = kn_m . qn_n               (keys on partitions, queries on free)
  P = exp(S^T)                          (logits bounded by |temp| => no max pass)
  [O; den] = [V | 1]^T @ P              (attn @ v with fused softmax denominator)
  partial = (w_out[:, 8h:8h+8] @ O) / den
Full output = sum over heads of partials (host-side reduce over the 8 cores).

Sharding: head h -> core h. Each core gets full X, Y and its head's weight
slices; returns a [64, 4096] partial of the final projection.
"""

import numpy as np

import concourse.bass as bass
import concourse.tile as tile
from concourse import mybir
from concourse.masks import make_identity
from concourse.vector_clock import ScopedClock

NUM_HEADS = 8
DIM = 64          # channels
HD = 8            # head dim
N = 4096          # tokens (h*w)
NB = 32           # 128-token blocks
QC = 512          # query chunk
NQC = N // QC
F32 = mybir.dt.float32
F32R = mybir.dt.float32r

# kb waves for the exp stage: 3 PSUM banks per wave (double buffered) so the
# activation engine runs one large-FD exp per wave.
WAVES = [list(range(s, min(s + 3, NB))) for s in range(0, NB, 3)]

import os
ROW_PACK_S = os.environ.get("KERN_ROW_PACK", "0") == "1"
COL_PACK_O = os.environ.get("KERN_COL_PACK", "0") == "1" 

_patched = False


def _apply_walrus_compat():
    """This container's walrus build rejects Drain instructions that carry
    sync waits ("Too many sync wait commands").  Replace multi-engine
    barriers with the sem-only variant and re-emit the TileContext tail
    drain's waits as standalone EventSemaphore instructions."""
    global _patched
    if _patched:
        return
    _patched = True

    def meb(self, engines):
        for e in engines:
            self.engines[e].drain()  # bare drain: flush pipelines, no waits
        for inst in self._sem_only_all_engine_barrier_insts("meb"):
            self.engines[inst.engine].add_instruction(inst)

    bass.Bass.multi_engine_barrier = meb

    def _drain_and_barrier(self, tick_clock, wait_clock):
        nc = self.nc
        carrier = nc.sync.nop()
        wait_clock.add_sem_waits(
            carrier.ins, ScopedClock({None: tick_clock.global_clock})
        )
        si = carrier.ins.sync_info
        waits = list(si.on_wait) if si and si.on_wait else []
        if si is not None:
            si.on_wait = []
        sems = list(self.sems.allocated().values())
        placeholder = sems[0] if sems else nc.alloc_semaphore("tailw")
        for w in waits:
            assert w.wait_mode in ("sem-ge-imm", "sem-ge"), w.wait_mode
            ev = nc.sync.wait_ge(placeholder, 0)
            ev.ins.sync_info.on_wait = [w]
        nc.sync.drain()
        nc.all_engine_barrier()
        popped = nc._tile_sem_poison_stack.pop()
        assert popped is self._sem_poison
        nc.clear_and_free_semaphores(list(self.sems.allocated().values()))
        nc.all_engine_barrier()

    tile.TileContext._drain_and_barrier = _drain_and_barrier

    # This walrus build allows at most ONE sync-wait command per instruction
    # (and none on Drain).  Split extra waits into standalone single-wait
    # EventSemaphore instructions emitted just before, on the same engine.
    orig_commit = tile.TileContext._commit_instruction

    def _commit_instruction(self, inst, lazy_reg_writes=True):
        si = inst.sync_info
        if si is not None and si.on_wait:
            is_drain = type(inst).__name__ == "InstDrain"
            waits = list(si.on_wait)
            n_ge = sum(
                1 for w in waits if w.wait_mode in ("sem-ge-imm", "sem-ge")
            )
            assert n_ge == len(waits) or not is_drain, f"eq-wait on drain {inst}"
            keep = 0 if is_drain else 1
            if len(waits) > keep and inst.engine != mybir.EngineType.Unassigned:
                kept, split = waits[:keep], waits[keep:]
                si.on_wait = kept
                sems = list(self.sems.allocated().values())
                placeholder = sems[0] if sems else self.nc.alloc_semaphore("splitw")
                eng = self.nc.engines[inst.engine]
                for w in split:
                    assert w.wait_mode in ("sem-ge-imm", "sem-ge"), w.wait_mode
                    ev = eng.wait_ge(placeholder, 0)
                    ev.ins.sync_info.on_wait = [w]
        return orig_commit(self, inst, lazy_reg_writes)

    tile.TileContext._commit_instruction = _commit_instruction


def _emit_head_attention(tc, rep, x_d, y_d, wqkvt_d, wot_d, temp_d, out_d):
    """Emit one head's full attention for one repetition."""
    import contextlib

    nc = tc.nc
    Exp = mybir.ActivationFunctionType.Exp
    Sqrt = mybir.ActivationFunctionType.Sqrt

    ctx = contextlib.ExitStack()
    with ctx:
        const = ctx.enter_context(tc.tile_pool(name=f"const{rep}", bufs=1))
        sb = ctx.enter_context(tc.tile_pool(name=f"sb{rep}", bufs=1))

        # ---- load inputs ----
        X = const.tile([DIM, N], F32)
        Y = const.tile([DIM, N], F32)
        W = const.tile([DIM, 3 * HD], F32)
        WOT = const.tile([128, DIM + 1], F32)
        WOTR = const.tile([128, DIM + 1], F32R)
        SKIP_IO = os.environ.get("KERN_SKIP_IO", "0") == "1"
        if SKIP_IO:
            nc.vector.memset(X[:], 0.5)
            nc.vector.memset(Y[:], 0.5)
            nc.vector.memset(W[:], 0.1)
            nc.vector.memset(WOT[:], 0.1)
        else:
            nc.sync.dma_start(X[:], x_d[:])
            nc.sync.dma_start(Y[:], y_d[:])
            nc.sync.dma_start(W[:], wqkvt_d[:])
            nc.sync.dma_start(WOT[:], wot_d[:])
        nc.vector.tensor_copy(WOTR[:], WOT[:])
        tmp_bc = const.tile([128, 1], F32)
        nc.gpsimd.dma_start(
            out=tmp_bc[:],
            in_=bass.AP(
                tensor=temp_d.tensor, offset=temp_d.offset, ap=[[0, 128], [1, 1]]
            ),
        )
        ident = const.tile([128, 128], F32)
        make_identity(nc, ident[:])

        # persistent SBUF state for this head
        QK = sb.tile([128, NB, 2 * HD], F32)   # token-major q|k
        Vaug = sb.tile([128, NB, HD + 1], F32R)  # token-major v | ones
        # channel-major normalized q/k, replicated at partitions 0/32/64 for
        # row-group-packed S matmuls
        Qcm = sb.tile([72, N], F32R)
        Kcm = sb.tile([72, N], F32R)
        rqk = sb.tile([128, 2 * NB], F32)      # per-token 1/norm (q | k)

        ones32 = const.tile([128, NB], F32)
        nc.vector.memset(ones32[:], 1.0)
        nc.vector.tensor_copy(
            Vaug[:, :, HD : HD + 1], ones32[:].rearrange("p (a b) -> p a b", b=1)
        )

        SKIP_PRE = os.environ.get("KERN_SKIP_PRE", "0") == "1"
        if SKIP_PRE:
            nc.vector.memset(QK[:].bitcast(F32), 0.5)
            nc.vector.memset(Vaug[:].bitcast(F32), 0.5)
            nc.vector.memset(Qcm[:].bitcast(F32), 0.25)
            nc.vector.memset(Kcm[:].bitcast(F32), 0.25)
        # ---- step 1: token-major projections  psum[128t, 24] per block ----
        with tc.tile_pool(name=f"pproj{rep}", bufs=4, space="PSUM") as pproj:
            for i in range(NB if not SKIP_PRE else 0):
                ps = pproj.tile([128, 3 * HD], F32)
                nc.tensor.matmul(
                    ps[:, 0:HD],
                    lhsT=X[:, i * 128 : (i + 1) * 128],
                    rhs=W[:, 0:HD],
                    start=True,
                    stop=True,
                )
                nc.tensor.matmul(
                    ps[:, HD : 3 * HD],
                    lhsT=Y[:, i * 128 : (i + 1) * 128],
                    rhs=W[:, HD : 3 * HD],
                    start=True,
                    stop=True,
                )
                nc.vector.tensor_copy(QK[:, i, :], ps[:, 0 : 2 * HD])
                nc.vector.tensor_copy(Vaug[:, i, 0:HD], ps[:, 2 * HD : 3 * HD])

        # ---- step 2: per-token L2 norms (token-major) ----
        sq = sb.tile([128, NB, HD], F32)
        nrm = sb.tile([128, 2 * NB], F32)
        if SKIP_PRE:
            nc.vector.memset(rqk[:], 1.0)
        nc.vector.tensor_mul(sq[:], QK[:, :, 0:HD], QK[:, :, 0:HD])
        nc.vector.tensor_reduce(
            nrm[:, 0:NB], sq[:], axis=mybir.AxisListType.X, op=mybir.AluOpType.add
        )
        nc.vector.tensor_mul(sq[:], QK[:, :, HD : 2 * HD], QK[:, :, HD : 2 * HD])
        nc.vector.tensor_reduce(
            nrm[:, NB : 2 * NB],
            sq[:],
            axis=mybir.AxisListType.X,
            op=mybir.AluOpType.add,
        )
        # norm = sqrt(ssq); rqk = 1/norm; fold temperature into q's.
        # (reference clamps the norm at 1e-12 — unreachable for randn data)
        nc.scalar.activation(nrm[:], nrm[:], Sqrt, bias=0.0)
        nc.vector.reciprocal(rqk[:], nrm[:])
        nc.vector.tensor_scalar_mul(rqk[:, 0:NB], in0=rqk[:, 0:NB], scalar1=tmp_bc[:])

        # ---- step 3: normalize q, k in token-major layout ----
        for i in range(NB):
            nc.vector.tensor_scalar_mul(
                QK[:, i, 0:HD], in0=QK[:, i, 0:HD], scalar1=rqk[:, i : i + 1]
            )
            nc.vector.tensor_scalar_mul(
                QK[:, i, HD : 2 * HD],
                in0=QK[:, i, HD : 2 * HD],
                scalar1=rqk[:, NB + i : NB + i + 1],
            )

        # ---- step 4: transpose to channel-major via the tensor engine ----
        with tc.tile_pool(name=f"ptr{rep}", bufs=4, space="PSUM") as ptr:
            for half, dst in ((0, Qcm), (1, Kcm)) if not SKIP_PRE else ():
                for g in range(NB // 4):  # 4 blocks -> one [8, 512] psum bank
                    pt = ptr.tile([HD, 512], F32)
                    for j in range(4):
                        i = g * 4 + j
                        nc.tensor.transpose(
                            pt[:, j * 128 : (j + 1) * 128],
                            QK[:, i, half * HD : (half + 1) * HD],
                            ident[:],
                        )
                    nc.vector.tensor_copy(
                        dst[0:HD, g * 512 : (g + 1) * 512], pt[:]
                    )

        # replicate q/k to partitions 32-39 and 64-71 (row groups 1, 2)
        for t in (Qcm, Kcm):
            for base in (32, 64):
                nc.sync.dma_start(
                    t[base : base + HD, :].bitcast(F32),
                    t[0:HD, :].bitcast(F32),
                )

        # ---- main loop ----
        pS = ctx.enter_context(tc.tile_pool(name=f"pS{rep}", bufs=2, space="PSUM"))
        pO = ctx.enter_context(tc.tile_pool(name=f"pO{rep}", bufs=1, space="PSUM"))
        pF = ctx.enter_context(tc.tile_pool(name=f"pF{rep}", bufs=1, space="PSUM"))
        Ppool = ctx.enter_context(tc.tile_pool(name=f"P{rep}", bufs=2))
        epi = ctx.enter_context(tc.tile_pool(name=f"epi{rep}", bufs=2))
        dram = ctx.enter_context(
            tc.tile_pool(name=f"dram{rep}", bufs=2, space="DRAM")
        )

        for qc in range(int(os.environ.get("KERN_NQC", NQC))):
            O = pO.tile([128, QC], F32)
            # zero the rows between the 4 accumulator groups so the epilogue
            # copy reads fully initialized memory
            nc.vector.memset(O[:], 0.0)
            for wave in WAVES:
                nw = len(wave)
                S = pS.tile([128, 3 * QC], F32, tag="S")
                P = Ppool.tile([128, 3 * QC], F32R, tag="P")
                for j, kb in enumerate(wave):
                    b = 32 * j if ROW_PACK_S else 0  # row group j
                    nc.tensor.matmul(
                        S[:, j * QC : (j + 1) * QC],
                        lhsT=Kcm[b : b + HD, kb * 128 : (kb + 1) * 128],
                        rhs=Qcm[b : b + HD, qc * QC : (qc + 1) * QC],
                        start=True,
                        stop=True,
                    )
                nc.scalar.activation(
                    P[:, 0 : nw * QC], S[:, 0 : nw * QC], Exp
                )
                for j, kb in enumerate(wave):
                    g = (kb % 4) if COL_PACK_O else 0
                    nc.tensor.matmul(
                        O[32 * g : 32 * g + HD + 1, :],
                        lhsT=Vaug[:, kb, :],
                        rhs=P[:, j * QC : (j + 1) * QC],
                        start=(kb < 4) if COL_PACK_O else (kb == 0),
                        stop=(kb >= NB - 4) if COL_PACK_O else (kb == NB - 1),
                        tile_position=(0, 32 * g) if COL_PACK_O else None,
                        skip_group_check=True,
                    )

            # epilogue: one matmul projects + merges the 4 accumulators and
            # sums the denominator rows (WOT4 col 64 selects them)
            O_sb = epi.tile([128, QC], F32R, tag="O_sb")
            nc.vector.tensor_copy(O_sb[:], O[:])
            proj = pF.tile([DIM + 1, QC], F32)
            nc.tensor.matmul(
                proj[:],
                lhsT=WOTR[:],
                rhs=O_sb[:],
                start=True,
                stop=True,
            )
            rden0 = epi.tile([DIM + 1, QC], F32, tag="rden0")
            nc.vector.reciprocal(rden0[DIM : DIM + 1, :], proj[DIM : DIM + 1, :])
            dscr = dram.tile([1, QC], F32, tag="dscr")
            nc.sync.dma_start(dscr[:], rden0[DIM : DIM + 1, :])
            rden = epi.tile([DIM, QC], F32, tag="rden")
            d0 = dscr[:]
            nc.sync.dma_start(
                rden[:],
                bass.AP(tensor=d0.tensor, offset=d0.offset, ap=[[0, DIM], [1, QC]]),
            )
            res = epi.tile([DIM, QC], F32, tag="res")
            nc.vector.tensor_mul(res[:], proj[0:DIM, :], rden[:])
            nc.sync.dma_start(out_d[:, qc * QC : (qc + 1) * QC], res[:])


def build_program(reps: int = 1):
    """Build the SPMD bass program (identical on all cores)."""
    _apply_walrus_compat()
    nc = bass.Bass("TRN2", target_bir_lowering=False, debug=False)
    x_d = nc.dram_tensor("x", [DIM, N], F32, kind="ExternalInput").ap()
    y_d = nc.dram_tensor("y", [DIM, N], F32, kind="ExternalInput").ap()
    wqkvt_d = nc.dram_tensor("wqkvt", [DIM, 3 * HD], F32, kind="ExternalInput").ap()
    wot_d = nc.dram_tensor("wot", [128, DIM + 1], F32, kind="ExternalInput").ap()
    temp_d = nc.dram_tensor("temp", [1, 1], F32, kind="ExternalInput").ap()
    outs = []
    with tile.TileContext(nc) as tc:
        for rep in range(reps):
            out_d = nc.dram_tensor(
                f"out{rep}", [DIM, N], F32, kind="ExternalOutput"
            ).ap()
            outs.append(f"out{rep}")
            _emit_head_attention(tc, rep, x_d, y_d, wqkvt_d, wot_d, temp_d, out_d)
    return nc, outs


def make_in_maps(x, y, w_q, w_kv, w_out, temperature):
    x = np.ascontiguousarray(np.asarray(x, dtype=np.float32))
    y = np.ascontiguousarray(np.asarray(y, dtype=np.float32))
    w_q = np.asarray(w_q, dtype=np.float32)
    w_kv = np.asarray(w_kv, dtype=np.float32)
    w_out = np.asarray(w_out, dtype=np.float32)
    temperature = np.asarray(temperature, dtype=np.float32)
    assert x.shape == (1, DIM, 64, 64) and y.shape == (1, DIM, 64, 64)
    X = x.reshape(DIM, N)
    Y = y.reshape(DIM, N)
    in_maps = []
    for h in range(NUM_HEADS):
        sl = slice(h * HD, (h + 1) * HD)
        wqkvt = np.concatenate(
            [w_q[sl].T, w_kv[sl].T, w_kv[DIM + h * HD : DIM + (h + 1) * HD].T],
            axis=1,
        )
        # cols 0..64 project the 4 accumulator groups; col 64 sums their
        # denominator rows
        wot4 = np.zeros((128, DIM + 1), dtype=np.float32)
        for g in range(4):
            wot4[32 * g : 32 * g + HD, 0:DIM] = w_out[:, sl].T
            wot4[32 * g + HD, DIM] = 1.0
        in_maps.append(
            {
                "x": X,
                "y": Y,
                "wqkvt": np.ascontiguousarray(wqkvt),
                "wot": wot4,
                "temp": temperature.reshape(NUM_HEADS)[h].reshape(1, 1),
            }
        )
    return in_maps


def kernel(x, y, w_q, w_kv, w_out, temperature):
    from concourse.bass_utils import run_bass_kernel_spmd

    nc, out_names = build_program(reps=1)
    in_maps = make_in_maps(x, y, w_q, w_kv, w_out, temperature)
    res = run_bass_kernel_spmd(nc, in_maps, list(range(NUM_HEADS)))
    total = np.zeros((DIM, N), dtype=np.float32)
    for h in range(NUM_HEADS):
        total += res.results[h][out_names[0]]
    return total.reshape(1, DIM, 64, 64)
